# revision 8
# baseline (speedup 1.0000x reference)
"""3-layer GAT on 8 trn2 NeuronCores (Bass/Tile).

Strategy (dst-node sharding, v2):
- N padded to 50176 = 392 dst-blocks x 128. Blocks are load-balanced across
  cores: blocks sorted by edge count, rank r -> core r%8, slot r//8; node ids
  below are in the permuted (core-major shard) space.
- Per layer a node table holds per-node rows [h bf16 | el f32 | er f32]
  (768 B for 256-dim layers, 256 B for the 64-dim layer-3 input). The dense
  phase is sharded (bf16 matmuls on PE); one AllGather replicates the table.
- Edge phase processes blocks in chunks of G slots: per chunk ONE dma_gather
  pulls all lo-src rows (src < 32768) and one pulls the hi rows (int16 index
  limit forces the split), with per-slot tile counts baked as the max over
  the 8 cores. Per block: w = exp(leaky_relu(el_src + er_dst)), then a bf16
  one-hot Sel matmul segment-reduces [w | w*h] into PSUM, giving the softmax
  denominator and weighted sum together (max-subtraction cancels in the
  ratio; logits are O(1) so f32 exp is safe).
- The next layer's dense matmul for each block's 128 nodes is interleaved
  right after the block epilogue so it hides inside the gather stream.
"""

import os
import sys

sys.path.insert(0, "/opt/trn_rl_repo")

PHASES = int(os.environ.get("GAT_PHASES", "99"))

import numpy as np
import ml_dtypes

import concourse.bass as bass
import concourse.tile as tile
import concourse.mybir as mybir
from concourse import bacc
from concourse.bass_utils import run_bass_kernel_spmd

bf16 = mybir.dt.bfloat16
f32 = mybir.dt.float32
i16 = mybir.dt.int16
AF = mybir.ActivationFunctionType
ALU = mybir.AluOpType

NCORES = 8
P = 128
SPLIT = 32768
NEG_SLOPE = 0.2
H = 4
F = 64
D = H * F  # 256
ROW = 384  # bf16 cols per 256-dim table row (h 0:256 | el f32 | er f32 | pad)
ROW3 = 128  # bf16 cols per 64-dim table row
NCHUNK = 13  # gather chunks per core (snake-assigned slots)


def _wrap_idx(arr):
    """[K] int16 (K%128==0) -> [128, K//16] dma_gather index layout."""
    k = arr.shape[0]
    a = arr.reshape(k // 16, 16).T  # [16, K/16]
    return np.ascontiguousarray(np.tile(a, (8, 1)))


def _col_layout(arr):
    """[T*128] -> [128, T]: slot t*128+p at (p, t)."""
    t = arr.shape[0] // 128
    return np.ascontiguousarray(arr.reshape(t, 128).T)


def _block_diag(a):
    h, f = a.shape
    out = np.zeros((h * f, h), np.float32)
    for i in range(h):
        out[i * f : (i + 1) * f, i] = a[i]
    return out


def kernel(feat, src, dst, W1, al1, ar1, b1, W2, al2, ar2, b2, W3, al3, ar3, b3):
    feat = np.asarray(feat, np.float32)
    src = np.asarray(src).astype(np.int64)
    dst = np.asarray(dst).astype(np.int64)
    params = [np.asarray(p, np.float32) for p in (W1, al1, ar1, b1, W2, al2, ar2, b2, W3, al3, ar3, b3)]
    W1, al1, ar1, b1, W2, al2, ar2, b2, W3, al3, ar3, b3 = params
    assert abs(b1).max() == 0 and abs(b2).max() == 0 and abs(b3).max() == 0

    N, DIN = feat.shape
    assert DIN == P
    NBLK = -(-(-(-N // P)) // NCORES) * NCORES  # 392
    NPAD = NBLK * P  # 50176
    BPC = NBLK // NCORES  # 49
    SHARD = BPC * P  # 6272

    # ---- host: block -> (core, slot) balanced assignment ----
    oblk = dst // P
    ocounts = np.bincount(oblk, minlength=NBLK)
    rank = np.argsort(-ocounts, kind="stable")  # rank r -> old block id
    blk_core = np.empty(NBLK, np.int64)
    blk_slot = np.empty(NBLK, np.int64)
    blk_core[rank] = np.arange(NBLK) % NCORES
    blk_slot[rank] = np.arange(NBLK) // NCORES
    nodes = np.arange(NPAD, dtype=np.int64)
    node_perm = blk_core[nodes // P] * SHARD + blk_slot[nodes // P] * P + nodes % P
    src_p = node_perm[src]
    ecore = blk_core[oblk]
    eslot = blk_slot[oblk]
    dloc = dst % P

    lo_lists = [[None] * BPC for _ in range(NCORES)]
    hi_lists = [[None] * BPC for _ in range(NCORES)]
    order = np.lexsort((src_p, eslot, ecore))
    src_s, dl_s = src_p[order], dloc[order]
    ec_s, es_s = ecore[order], eslot[order]
    bounds = np.searchsorted(ec_s * BPC + es_s, np.arange(NCORES * BPC + 1))
    for c in range(NCORES):
        for j in range(BPC):
            s, e = bounds[c * BPC + j], bounds[c * BPC + j + 1]
            ss, dd = src_s[s:e], dl_s[s:e]
            nl = int(np.searchsorted(ss, SPLIT))
            lo_lists[c][j] = (ss[:nl].astype(np.int16), dd[:nl])
            hi_lists[c][j] = ((ss[nl:] - SPLIT).astype(np.int16), dd[nl:])

    tlo = [max(max(-(-len(lo_lists[c][j][0]) // P), 1) for c in range(NCORES)) for j in range(BPC)]
    thi = [max(max(-(-len(hi_lists[c][j][0]) // P), 1) for c in range(NCORES)) for j in range(BPC)]
    T = [tlo[j] + thi[j] for j in range(BPC)]
    TMAX = max(T)

    # snake chunks: chunk ci gets slots ci, ci+13, ci+26, ci+39 (bounded)
    chunks = [[j for j in range(ci, BPC, NCHUNK)] for ci in range(NCHUNK)]
    CTLO = [sum(tlo[j] for j in ch) for ch in chunks]
    CTHI = [sum(thi[j] for j in ch) for ch in chunks]
    CT = [a + b for a, b in zip(CTLO, CTHI)]
    CTMAX = max(CT)

    # ---- host: per-core idx / dstloc constants ----
    def core_consts(c):
        lo_cols, hi_cols, dl_cols = [], [], []
        for ch in chunks:
            for j in ch:
                a = np.zeros(tlo[j] * P, np.int16)
                v = lo_lists[c][j][0]
                a[: len(v)] = v
                lo_cols.append(_wrap_idx(a))
            for j in ch:
                a = np.zeros(thi[j] * P, np.int16)
                v = hi_lists[c][j][0]
                a[: len(v)] = v
                hi_cols.append(_wrap_idx(a))
        for j in range(BPC):
            dlo = lo_lists[c][j][1]
            dhi = hi_lists[c][j][1]
            a = np.full(T[j] * P, -1.0, np.float32)
            a[: len(dlo)] = dlo
            a[tlo[j] * P : tlo[j] * P + len(dhi)] = dhi
            dl_cols.append(_col_layout(a.astype(ml_dtypes.bfloat16).view(np.int16)))
        return (
            np.concatenate(lo_cols, axis=1),
            np.concatenate(hi_cols, axis=1),
            np.concatenate(dl_cols, axis=1),
        )

    # ---- host: weights (bf16) ----
    def wall(W, al, ar):
        wel = W @ _block_diag(al)
        wer = W @ _block_diag(ar)
        return (
            np.concatenate([W, wel, wer], axis=1)
            .astype(ml_dtypes.bfloat16)
            .view(np.int16)
        )

    wall1 = wall(W1, al1, ar1)  # [128, 264]
    wall2 = wall(W2, al2, ar2)  # [256, 264]
    wall3 = wall(W3, al3, ar3)  # [256, 66]
    NW = D + 2 * H  # 264
    NW3 = F + 2  # 66

    iota_np = np.tile(np.arange(P, dtype=np.float32), (P, 1)).astype(ml_dtypes.bfloat16)
    idnb_np = np.eye(P, dtype=np.float32).astype(ml_dtypes.bfloat16)

    def blob_for_core(c):
        lo_i, hi_i, dl_i = core_consts(c)
        fields = [
            ("iota", iota_np.view(np.int16)),
            ("idnb", idnb_np.view(np.int16)),
            ("wall1", wall1),
            ("wall2k0", wall2[0:P]),
            ("wall2k1", wall2[P : 2 * P]),
            ("wall3k0", wall3[0:P]),
            ("wall3k1", wall3[P : 2 * P]),
            ("lo", lo_i),
            ("hi", hi_i),
            ("dstloc", dl_i),
        ]
        blob = np.concatenate([f for _, f in fields], axis=1)
        if blob.shape[1] % 2:
            blob = np.concatenate([blob, np.zeros((P, 1), np.int16)], axis=1)
        offs = {}
        o = 0
        for name, f in fields:
            offs[name] = o
            o += f.shape[1]
        return np.ascontiguousarray(blob), offs

    blob0, offs = blob_for_core(0)
    CB = blob0.shape[1]

    lo_coff, hi_coff = [], []
    o1 = o2 = 0
    for ci in range(NCHUNK):
        lo_coff.append(o1)
        hi_coff.append(o2)
        o1 += CTLO[ci] * P // 16
        o2 += CTHI[ci] * P // 16
    dl_off = np.zeros(BPC + 1, np.int64)
    np.cumsum(T, out=dl_off[1:])

    featT = np.zeros((P, NPAD), np.float32)
    featT[:, node_perm[np.arange(N)]] = feat.T
    featT = featT.astype(ml_dtypes.bfloat16)

    # ---- build program ----
    nc = bacc.Bacc("TRN2", target_bir_lowering=False, debug=False, num_devices=NCORES)

    cblob_in = nc.dram_tensor("cblob", [P, CB], i16, kind="ExternalInput")
    featT_in = nc.dram_tensor("featT", [P, SHARD], bf16, kind="ExternalInput")
    out_ext = nc.dram_tensor("out", [SHARD, F], f32, kind="ExternalOutput")

    tab1_sh = nc.dram_tensor("tab1_sh", [SHARD, ROW], bf16)
    tab2_sh = nc.dram_tensor("tab2_sh", [SHARD, ROW], bf16)
    tab3_sh = nc.dram_tensor("tab3_sh", [SHARD, ROW3], bf16)
    tab1 = nc.dram_tensor("tab1", [NPAD, ROW], bf16, addr_space="Shared")
    tab2 = nc.dram_tensor("tab2", [NPAD, ROW], bf16, addr_space="Shared")
    tab3 = nc.dram_tensor("tab3", [NPAD, ROW3], bf16, addr_space="Shared")

    rg = [list(range(NCORES))]

    with tile.TileContext(nc) as tc:
        with (
            tc.tile_pool(name="const", bufs=1) as cp,
            tc.tile_pool(name="hxp", bufs=2) as hxp,
            tc.tile_pool(name="work", bufs=2) as wp,
            tc.tile_pool(name="small", bufs=2) as sp,
            tc.tile_pool(name="psum", bufs=2, space="PSUM") as pp,
        ):
            cblob = cp.tile([P, CB], i16)
            nc.sync.dma_start(cblob[:], cblob_in[:])
            iota = cblob[:, offs["iota"] : offs["iota"] + P].bitcast(bf16)
            idnb = cblob[:, offs["idnb"] : offs["idnb"] + P].bitcast(bf16)
            wall1_t = cblob[:, offs["wall1"] : offs["wall1"] + NW].bitcast(bf16)
            wall2_t = [
                cblob[:, offs[f"wall2k{k}"] : offs[f"wall2k{k}"] + NW].bitcast(bf16)
                for k in range(2)
            ]
            wall3_t = [
                cblob[:, offs[f"wall3k{k}"] : offs[f"wall3k{k}"] + NW3].bitcast(bf16)
                for k in range(2)
            ]

            kregs = {}
            for ci in range(NCHUNK):
                for k in (CTLO[ci] * P, CTHI[ci] * P):
                    if k not in kregs:
                        kregs[k] = nc.gpsimd.to_reg(k)

            def dense_write(x_ap, j, wall_k, nw, tab_shard, row_cols, hsz, first):
                """dense for the 128 nodes of slot j -> table rows
                [h bf16 | el er f32]. x_ap: [128, nk*128] bf16 node-major."""
                psd = pp.tile([P, NW], f32, tag="psd", space="PSUM")
                nk = len(wall_k)
                if first is not None:
                    nc.tensor.matmul(psd[:, :nw], first, wall_k[0][:, :nw], start=True, stop=True)
                else:
                    lhsT = sp.tile([P, 2, P], bf16, tag="lhsT")
                    for k in range(nk):
                        ptr = pp.tile([P, P], bf16, tag="ptr", space="PSUM")
                        nc.tensor.transpose(ptr[:], x_ap[:, k * P : (k + 1) * P], idnb)
                        nc.vector.tensor_copy(lhsT[:, k, :], ptr[:])
                    for k in range(nk):
                        nc.tensor.matmul(
                            psd[:, :nw],
                            lhsT[:, k, :],
                            wall_k[k][:, :nw],
                            start=(k == 0),
                            stop=(k == nk - 1),
                        )
                row = sp.tile([P, ROW], bf16, tag="row")
                nc.vector.tensor_copy(row[:, 0:hsz], psd[:, 0:hsz])
                nc.vector.tensor_copy(
                    row[:, hsz : hsz + 2 * (nw - hsz)].bitcast(f32),
                    psd[:, hsz:nw],
                )
                nc.sync.dma_start(tab_shard[j * P : (j + 1) * P, :], row[:, 0:row_cols])

            # ---- dense layer 1 ----
            for j in range(BPC):
                ft = sp.tile([P, P], bf16, tag="ft")
                nc.sync.dma_start(ft[:], featT_in[:, j * P : (j + 1) * P])
                dense_write(None, j, [wall1_t], NW, tab1_sh, ROW, D, first=ft[:])

            if PHASES >= 2:
                nc.gpsimd.collective_compute(
                    "AllGather", ALU.bypass, replica_groups=rg, ins=[tab1_sh[:]], outs=[tab1[:]]
                )

            # ---- edge phase for one layer ----
            def edge_layer(tab_full, tab_shard, row, heads, hsz, nxt):
                nmsg = heads + hsz
                for ci in range(NCHUNK):
                    ch = chunks[ci]
                    ctlo, cthi, ct = CTLO[ci], CTHI[ci], CT[ci]
                    hx_raw = hxp.tile([P, CTMAX * ROW], bf16, tag="hx")
                    hx = hx_raw[:, 0 : ct * row].rearrange("p (t r) -> p t r", r=row)
                    nc.gpsimd.dma_gather(
                        hx[:, 0:ctlo, :],
                        tab_full[0:SPLIT],
                        cblob[:, offs["lo"] + lo_coff[ci] : offs["lo"] + lo_coff[ci] + ctlo * P // 16],
                        ctlo * P,
                        kregs[ctlo * P],
                        row,
                        elem_step=row,
                        single_packet=False,
                    )
                    nc.gpsimd.dma_gather(
                        hx[:, ctlo:ct, :],
                        tab_full[SPLIT:NPAD],
                        cblob[:, offs["hi"] + hi_coff[ci] : offs["hi"] + hi_coff[ci] + cthi * P // 16],
                        cthi * P,
                        kregs[cthi * P],
                        row,
                        elem_step=row,
                        single_packet=False,
                    )
                    lo_b, hi_b = 0, 0
                    for j in ch:
                        tl, th, tj = tlo[j], thi[j], T[j]
                        lo_sl = hx[:, lo_b : lo_b + tl, :]
                        hi_sl = hx[:, ctlo + hi_b : ctlo + hi_b + th, :]
                        lo_b += tl
                        hi_b += th
                        # er for the block's 128 dsts
                        erch = sp.tile([P, 128], bf16, tag="erch")
                        nc.sync.dma_start(
                            erch[:], tab_shard[j * P : (j + 1) * P, row - 128 : row]
                        )
                        eroff0 = 128 - (row - hsz)
                        er_blk = sp.tile([P, heads], bf16, tag=f"er_blk{heads}")
                        nc.scalar.activation(
                            er_blk[:],
                            erch[:, eroff0 + 2 * heads : eroff0 + 4 * heads].bitcast(f32),
                            AF.Copy,
                        )
                        # sel one-hot [e, d]
                        sel = wp.tile([P, TMAX, P], bf16, tag="sel")
                        do = offs["dstloc"] + int(dl_off[j])
                        nc.vector.tensor_tensor(
                            out=sel[:, 0:tj],
                            in0=cblob[:, do : do + tj]
                            .bitcast(bf16)
                            .unsqueeze(2)
                            .to_broadcast([P, tj, P]),
                            in1=iota.unsqueeze(1).to_broadcast([P, tj, P]),
                            op=ALU.is_equal,
                        )
                        # er per edge via selT matmul
                        er_ps = pp.tile([P, TMAX * H], f32, tag="erps", space="PSUM")
                        for t in range(tj):
                            selT_ps = pp.tile([P, P], bf16, tag="ptr", space="PSUM")
                            nc.tensor.transpose(selT_ps[:], sel[:, t, :], idnb)
                            selT = sp.tile([P, P], bf16, tag="selT_sb")
                            nc.scalar.activation(selT[:], selT_ps[:], AF.Copy)
                            nc.tensor.matmul(
                                er_ps[:, t * heads : (t + 1) * heads],
                                selT[:],
                                er_blk[:],
                                start=True,
                                stop=True,
                            )
                        # e = el[src] + er[dst]; w = exp(lrelu(e))
                        e_t = sp.tile([P, TMAX, heads], f32, tag=f"e_t{heads}")
                        for sl, t0, tn in ((lo_sl, 0, tl), (hi_sl, tl, th)):
                            nc.vector.tensor_tensor(
                                out=e_t[:, t0 : t0 + tn],
                                in0=sl[:, :, hsz : hsz + 2 * heads].bitcast(f32),
                                in1=er_ps[:, t0 * heads : (t0 + tn) * heads].rearrange(
                                    "p (t h) -> p t h", h=heads
                                ),
                                op=ALU.add,
                            )
                        lr = sp.tile([P, TMAX, heads], f32, tag=f"lr{heads}")
                        nc.vector.tensor_scalar_mul(lr[:, 0:tj], e_t[:, 0:tj], NEG_SLOPE)
                        nc.vector.tensor_tensor(
                            out=lr[:, 0:tj], in0=e_t[:, 0:tj], in1=lr[:, 0:tj], op=ALU.max
                        )
                        msg = wp.tile([P, TMAX, nmsg], bf16, tag=f"msg{nmsg}")
                        nc.scalar.activation(msg[:, 0:tj, 0:heads], lr[:, 0:tj], AF.Exp)
                        for sl, t0, tn in ((lo_sl, 0, tl), (hi_sl, tl, th)):
                            nc.vector.tensor_tensor(
                                out=msg[:, t0 : t0 + tn, heads:nmsg],
                                in0=sl[:, :, 0:hsz],
                                in1=msg[:, t0 : t0 + tn, 0:heads]
                                .unsqueeze(3)
                                .to_broadcast([P, tn, heads, F]),
                                op=ALU.mult,
                            )
                        # segment-reduce into PSUM
                        ps_raw = pp.tile([P, H + D], f32, tag="agg", space="PSUM")
                        ps = ps_raw[:, 0:nmsg]
                        for t in range(tj):
                            nc.tensor.matmul(
                                ps,
                                sel[:, t, :],
                                msg[:, t, :],
                                start=(t == 0),
                                stop=(t == tj - 1),
                            )
                        # epilogue: out = act(wh_sum / w_sum)
                        rcp = sp.tile([P, 2, heads], f32, tag=f"rcp{heads}")
                        nc.vector.tensor_scalar(
                            out=rcp[:, 0, :], in0=ps[:, 0:heads], scalar1=1e-30,
                            scalar2=None, op0=ALU.max,
                        )
                        nc.vector.reciprocal(rcp[:, 1, :], rcp[:, 0, :])
                        x_sb = sp.tile([P, hsz], bf16, tag=f"x{hsz}")
                        nc.vector.tensor_tensor(
                            out=x_sb[:].rearrange("p (h f) -> p h f", h=heads),
                            in0=ps[:, heads:nmsg].rearrange("p (h f) -> p h f", h=heads),
                            in1=rcp[:, 1, :].unsqueeze(2).to_broadcast([P, heads, F]),
                            op=ALU.mult,
                        )
                        if nxt != "out":
                            nc.vector.tensor_scalar_max(x_sb[:], x_sb[:], 0.0)
                            wall_k, nw, tab_sh_n, row_n, hsz_n = nxt
                            dense_write(x_sb[:], j, wall_k, nw, tab_sh_n, row_n, hsz_n, None)
                        else:
                            xf = sp.tile([P, F], f32, tag="xf")
                            nc.vector.tensor_copy(xf[:], x_sb[:, 0:F])
                            nc.sync.dma_start(out_ext[j * P : (j + 1) * P, :], xf[:])

            if PHASES >= 3:
                edge_layer(tab1, tab1_sh, ROW, H, D, (wall2_t, NW, tab2_sh, ROW, D))
            if PHASES >= 4:
                nc.gpsimd.collective_compute(
                    "AllGather", ALU.bypass, replica_groups=rg, ins=[tab2_sh[:]], outs=[tab2[:]]
                )
                edge_layer(tab2, tab2_sh, ROW, H, D, (wall3_t, NW3, tab3_sh, ROW3, F))
            if PHASES >= 5:
                nc.gpsimd.collective_compute(
                    "AllGather", ALU.bypass, replica_groups=rg, ins=[tab3_sh[:]], outs=[tab3[:]]
                )
                edge_layer(tab3, tab3_sh, ROW3, 1, F, "out")

    nc.compile()

    in_maps = []
    for c in range(NCORES):
        blob_c = blob0 if c == 0 else blob_for_core(c)[0]
        in_maps.append(
            {
                "cblob": blob_c,
                "featT": np.ascontiguousarray(featT[:, c * SHARD : (c + 1) * SHARD]),
            }
        )

    trace = os.environ.get("GAT_TRACE", "0") == "1"
    if trace and "antenv.axon_hooks" not in sys.modules:
        import types

        _mod = types.ModuleType("antenv.axon_hooks")
        _mod._hook = None
        _mod.set_axon_ntff_profile_hook = lambda h: setattr(_mod, "_hook", h)
        _mod.get_axon_ntff_profile_hook = lambda: _mod._hook
        sys.modules["antenv.axon_hooks"] = _mod
        try:
            from trn_agent_boot.trn_boot import _ntff_profile_via_ctypes

            _mod._hook = _ntff_profile_via_ctypes("/opt/axon/libaxon_pjrt.so")
        except Exception as _e:
            print(f"ntff hook setup failed: {type(_e).__name__}: {_e}")
    res = None
    for attempt in range(4):
        try:
            res = run_bass_kernel_spmd(
                nc, in_maps, list(range(NCORES)), trace=trace and attempt < 2
            )
            break
        except Exception:
            if attempt == 3:
                raise
            import time

            time.sleep(20 * (attempt + 1))
    if trace:
        print(f"HW exec time: {res.exec_time_ns} ns")
        if res.instructions_and_trace is not None:
            print(f"trace path: {res.instructions_and_trace[1]}")
        if res.profile_json is not None:
            print(f"profile json: {res.profile_json}")
        global LAST_RESULTS
        LAST_RESULTS = res
    out_p = np.concatenate([res.results[c]["out"] for c in range(NCORES)], axis=0)
    out = np.ascontiguousarray(out_p[node_perm[np.arange(N)]]).astype(np.float32)
    return out


# revision 9
# speedup vs baseline: 1.2576x; 1.2576x over previous
"""3-layer GAT on 8 trn2 NeuronCores (Bass/Tile).

Strategy (dst-node sharding, v3):
- N padded to 50176 = 392 dst-blocks x 128. Blocks are load-balanced across
  cores (sorted by edge count, rank r -> core r%8, slot r//8); node ids are
  in the permuted core-major shard space.
- Per layer a node table holds per-node rows [h bf16 | el f32 | er f32]
  (768 B for 256-dim layers, 256 B for the 64-dim layer-3 input). The dense
  phase is sharded (bf16 matmuls on PE); one AllGather replicates the table.
- Edge phase: blocks processed in chunks of 2 slots; per chunk one dma_gather
  pulls the lo-src rows (src < 32768) and one the hi rows (int16 idx limit),
  per-slot tile counts baked as the max over the 8 cores. The per-edge ops
  (sel one-hot build, el+er add, leaky-relu, exp, w*h) are batched across
  the whole chunk; a bf16 one-hot Sel matmul then segment-reduces [w | w*h]
  per block into PSUM (softmax max-subtraction cancels in the ratio).
- The next layer's dense matmul for each block is interleaved after the
  block epilogue so it hides inside the gather stream.
"""

import os
import sys

sys.path.insert(0, "/opt/trn_rl_repo")

PHASES = int(os.environ.get("GAT_PHASES", "99"))

import numpy as np
import ml_dtypes

import concourse.bass as bass
import concourse.tile as tile
import concourse.mybir as mybir
from concourse import bacc
from concourse.bass_utils import run_bass_kernel_spmd

bf16 = mybir.dt.bfloat16
f32 = mybir.dt.float32
i16 = mybir.dt.int16
AF = mybir.ActivationFunctionType
ALU = mybir.AluOpType

NCORES = 8
P = 128
SPLIT = 32768
NEG_SLOPE = 0.2
H = 4
F = 64
D = H * F  # 256
ROW = 384  # bf16 cols per 256-dim table row (h 0:256 | el f32 | er f32 | pad)
ROW3 = 128  # bf16 cols per 64-dim table row
NCHUNK = 25  # gather chunks per core (snake-assigned slots, <=2 slots each)


def _wrap_idx(arr):
    """[K] int16 (K%128==0) -> [128, K//16] dma_gather index layout."""
    k = arr.shape[0]
    a = arr.reshape(k // 16, 16).T  # [16, K/16]
    return np.ascontiguousarray(np.tile(a, (8, 1)))


def _col_layout(arr):
    """[T*128] -> [128, T]: slot t*128+p at (p, t)."""
    t = arr.shape[0] // 128
    return np.ascontiguousarray(arr.reshape(t, 128).T)


def _block_diag(a):
    h, f = a.shape
    out = np.zeros((h * f, h), np.float32)
    for i in range(h):
        out[i * f : (i + 1) * f, i] = a[i]
    return out


def kernel(feat, src, dst, W1, al1, ar1, b1, W2, al2, ar2, b2, W3, al3, ar3, b3):
    feat = np.asarray(feat, np.float32)
    src = np.asarray(src).astype(np.int64)
    dst = np.asarray(dst).astype(np.int64)
    params = [np.asarray(p, np.float32) for p in (W1, al1, ar1, b1, W2, al2, ar2, b2, W3, al3, ar3, b3)]
    W1, al1, ar1, b1, W2, al2, ar2, b2, W3, al3, ar3, b3 = params
    assert abs(b1).max() == 0 and abs(b2).max() == 0 and abs(b3).max() == 0

    N, DIN = feat.shape
    assert DIN == P
    NBLK = -(-(-(-N // P)) // NCORES) * NCORES  # 392
    NPAD = NBLK * P  # 50176
    BPC = NBLK // NCORES  # 49
    SHARD = BPC * P  # 6272

    # ---- host: block -> (core, slot) balanced assignment ----
    oblk = dst // P
    ocounts = np.bincount(oblk, minlength=NBLK)
    rank = np.argsort(-ocounts, kind="stable")
    blk_core = np.empty(NBLK, np.int64)
    blk_slot = np.empty(NBLK, np.int64)
    blk_core[rank] = np.arange(NBLK) % NCORES
    blk_slot[rank] = np.arange(NBLK) // NCORES
    nodes = np.arange(NPAD, dtype=np.int64)
    node_perm = blk_core[nodes // P] * SHARD + blk_slot[nodes // P] * P + nodes % P
    src_p = node_perm[src]
    ecore = blk_core[oblk]
    eslot = blk_slot[oblk]
    dloc = dst % P

    lo_lists = [[None] * BPC for _ in range(NCORES)]
    hi_lists = [[None] * BPC for _ in range(NCORES)]
    order = np.lexsort((src_p, eslot, ecore))
    src_s, dl_s = src_p[order], dloc[order]
    ec_s, es_s = ecore[order], eslot[order]
    bounds = np.searchsorted(ec_s * BPC + es_s, np.arange(NCORES * BPC + 1))
    for c in range(NCORES):
        for j in range(BPC):
            s, e = bounds[c * BPC + j], bounds[c * BPC + j + 1]
            ss, dd = src_s[s:e], dl_s[s:e]
            nl = int(np.searchsorted(ss, SPLIT))
            lo_lists[c][j] = (ss[:nl].astype(np.int16), dd[:nl])
            hi_lists[c][j] = ((ss[nl:] - SPLIT).astype(np.int16), dd[nl:])

    tlo = [max(max(-(-len(lo_lists[c][j][0]) // P), 1) for c in range(NCORES)) for j in range(BPC)]
    thi = [max(max(-(-len(hi_lists[c][j][0]) // P), 1) for c in range(NCORES)) for j in range(BPC)]
    T = [tlo[j] + thi[j] for j in range(BPC)]

    # snake chunks: chunk ci gets slots {ci, ci+25} (bounded by 49)
    chunks = [[j for j in range(ci, BPC, NCHUNK)] for ci in range(NCHUNK)]
    CTLO = [sum(tlo[j] for j in ch) for ch in chunks]
    CTHI = [sum(thi[j] for j in ch) for ch in chunks]
    CT = [a + b for a, b in zip(CTLO, CTHI)]
    CTMAX = max(CT)

    # chunk-tile order: [lo_s0 | lo_s1 | hi_s0 | hi_s1]; per-block tile lists
    blk_tiles = {}
    for ci, ch in enumerate(chunks):
        lo_b, hi_b = 0, 0
        for j in ch:
            tls = list(range(lo_b, lo_b + tlo[j])) + [
                CTLO[ci] + hi_b + t for t in range(thi[j])
            ]
            blk_tiles[j] = (ci, tls)
            lo_b += tlo[j]
            hi_b += thi[j]

    # ---- host: per-core idx / dstloc constants ----
    def core_consts(c):
        lo_cols, hi_cols, dl_cols = [], [], []
        for ch in chunks:
            for j in ch:
                a = np.zeros(tlo[j] * P, np.int16)
                v = lo_lists[c][j][0]
                a[: len(v)] = v
                lo_cols.append(_wrap_idx(a))
            for j in ch:
                a = np.zeros(thi[j] * P, np.int16)
                v = hi_lists[c][j][0]
                a[: len(v)] = v
                hi_cols.append(_wrap_idx(a))
            for which in (0, 1):  # dstloc in chunk-tile order: lo slots, hi slots
                for j in ch:
                    tl = tlo[j] if which == 0 else thi[j]
                    dd = lo_lists[c][j][1] if which == 0 else hi_lists[c][j][1]
                    a = np.full(tl * P, -1.0, np.float32)
                    a[: len(dd)] = dd
                    dl_cols.append(_col_layout(a.astype(ml_dtypes.bfloat16).view(np.int16)))
        return (
            np.concatenate(lo_cols, axis=1),
            np.concatenate(hi_cols, axis=1),
            np.concatenate(dl_cols, axis=1),
        )

    # ---- host: weights (bf16) ----
    def wall(W, al, ar):
        wel = W @ _block_diag(al)
        wer = W @ _block_diag(ar)
        return (
            np.concatenate([W, wel, wer], axis=1)
            .astype(ml_dtypes.bfloat16)
            .view(np.int16)
        )

    wall1 = wall(W1, al1, ar1)
    wall2 = wall(W2, al2, ar2)
    wall3 = wall(W3, al3, ar3)
    NW = D + 2 * H  # 264
    NW3 = F + 2  # 66

    iota_np = np.tile(np.arange(P, dtype=np.float32), (P, 1)).astype(ml_dtypes.bfloat16)
    idnb_np = np.eye(P, dtype=np.float32).astype(ml_dtypes.bfloat16)

    def blob_for_core(c):
        lo_i, hi_i, dl_i = core_consts(c)
        fields = [
            ("iota", iota_np.view(np.int16)),
            ("idnb", idnb_np.view(np.int16)),
            ("wall1", wall1),
            ("wall2k0", wall2[0:P]),
            ("wall2k1", wall2[P : 2 * P]),
            ("wall3k0", wall3[0:P]),
            ("wall3k1", wall3[P : 2 * P]),
            ("lo", lo_i),
            ("hi", hi_i),
            ("dstloc", dl_i),
        ]
        blob = np.concatenate([f for _, f in fields], axis=1)
        if blob.shape[1] % 2:
            blob = np.concatenate([blob, np.zeros((P, 1), np.int16)], axis=1)
        offs = {}
        o = 0
        for name, f in fields:
            offs[name] = o
            o += f.shape[1]
        return np.ascontiguousarray(blob), offs

    blob0, offs = blob_for_core(0)
    CB = blob0.shape[1]

    lo_coff, hi_coff, dl_coff = [], [], []
    o1 = o2 = o3 = 0
    for ci in range(NCHUNK):
        lo_coff.append(o1)
        hi_coff.append(o2)
        dl_coff.append(o3)
        o1 += CTLO[ci] * P // 16
        o2 += CTHI[ci] * P // 16
        o3 += CT[ci]

    featT = np.zeros((P, NPAD), np.float32)
    featT[:, node_perm[np.arange(N)]] = feat.T
    featT = featT.astype(ml_dtypes.bfloat16)

    # ---- build program ----
    nc = bacc.Bacc("TRN2", target_bir_lowering=False, debug=False, num_devices=NCORES)

    cblob_in = nc.dram_tensor("cblob", [P, CB], i16, kind="ExternalInput")
    featT_in = nc.dram_tensor("featT", [P, SHARD], bf16, kind="ExternalInput")
    out_ext = nc.dram_tensor("out", [SHARD, F], f32, kind="ExternalOutput")

    tab1_sh = nc.dram_tensor("tab1_sh", [SHARD, ROW], bf16)
    tab2_sh = nc.dram_tensor("tab2_sh", [SHARD, ROW], bf16)
    tab3_sh = nc.dram_tensor("tab3_sh", [SHARD, ROW3], bf16)
    tab1 = nc.dram_tensor("tab1", [NPAD, ROW], bf16, addr_space="Shared")
    tab2 = nc.dram_tensor("tab2", [NPAD, ROW], bf16, addr_space="Shared")
    tab3 = nc.dram_tensor("tab3", [NPAD, ROW3], bf16, addr_space="Shared")

    rg = [list(range(NCORES))]

    with tile.TileContext(nc) as tc:
        with (
            tc.tile_pool(name="const", bufs=1) as cp,
            tc.tile_pool(name="hxp", bufs=2) as hxp,
            tc.tile_pool(name="work", bufs=2) as wp,
            tc.tile_pool(name="small", bufs=2) as sp,
            tc.tile_pool(name="psum", bufs=2, space="PSUM") as pp,
        ):
            cblob = cp.tile([P, CB], i16)
            nc.sync.dma_start(cblob[:], cblob_in[:])
            iota = cblob[:, offs["iota"] : offs["iota"] + P].bitcast(bf16)
            idnb = cblob[:, offs["idnb"] : offs["idnb"] + P].bitcast(bf16)
            wall1_t = cblob[:, offs["wall1"] : offs["wall1"] + NW].bitcast(bf16)
            wall2_t = [
                cblob[:, offs[f"wall2k{k}"] : offs[f"wall2k{k}"] + NW].bitcast(bf16)
                for k in range(2)
            ]
            wall3_t = [
                cblob[:, offs[f"wall3k{k}"] : offs[f"wall3k{k}"] + NW3].bitcast(bf16)
                for k in range(2)
            ]

            kregs = {}
            for ci in range(NCHUNK):
                for k in (CTLO[ci] * P, CTHI[ci] * P):
                    if k not in kregs:
                        kregs[k] = nc.gpsimd.to_reg(k)

            def dense_write(x_ap, j, wall_k, nw, tab_shard, row_cols, hsz, first):
                psd = pp.tile([P, NW], f32, tag="psd", space="PSUM")
                nk = len(wall_k)
                if first is not None:
                    nc.tensor.matmul(psd[:, :nw], first, wall_k[0][:, :nw], start=True, stop=True)
                else:
                    lhsT = sp.tile([P, 2, P], bf16, tag="lhsT")
                    for k in range(nk):
                        ptr = pp.tile([P, P], bf16, tag="ptr", space="PSUM")
                        nc.tensor.transpose(ptr[:], x_ap[:, k * P : (k + 1) * P], idnb)
                        nc.vector.tensor_copy(lhsT[:, k, :], ptr[:])
                    for k in range(nk):
                        nc.tensor.matmul(
                            psd[:, :nw],
                            lhsT[:, k, :],
                            wall_k[k][:, :nw],
                            start=(k == 0),
                            stop=(k == nk - 1),
                        )
                row = sp.tile([P, ROW], bf16, tag="row")
                nc.vector.tensor_copy(row[:, 0:hsz], psd[:, 0:hsz])
                nc.vector.tensor_copy(
                    row[:, hsz : hsz + 2 * (nw - hsz)].bitcast(f32),
                    psd[:, hsz:nw],
                )
                nc.sync.dma_start(tab_shard[j * P : (j + 1) * P, :], row[:, 0:row_cols])

            # ---- dense layer 1 ----
            for j in range(BPC):
                ft = sp.tile([P, P], bf16, tag="ft")
                nc.sync.dma_start(ft[:], featT_in[:, j * P : (j + 1) * P])
                dense_write(None, j, [wall1_t], NW, tab1_sh, ROW, D, first=ft[:])

            if PHASES >= 2:
                nc.gpsimd.collective_compute(
                    "AllGather", ALU.bypass, replica_groups=rg, ins=[tab1_sh[:]], outs=[tab1[:]]
                )

            # ---- edge phase for one layer ----
            def edge_layer(tab_full, tab_shard, row, heads, hsz, nxt):
                nmsg = heads + hsz
                for ci in range(NCHUNK):
                    ch = chunks[ci]
                    ctlo, cthi, ct = CTLO[ci], CTHI[ci], CT[ci]
                    hx_raw = hxp.tile([P, CTMAX * ROW], bf16, tag="hx")
                    hx = hx_raw[:, 0 : ct * row].rearrange("p (t r) -> p t r", r=row)
                    nc.gpsimd.dma_gather(
                        hx[:, 0:ctlo, :],
                        tab_full[0:SPLIT],
                        cblob[:, offs["lo"] + lo_coff[ci] : offs["lo"] + lo_coff[ci] + ctlo * P // 16],
                        ctlo * P,
                        kregs[ctlo * P],
                        row,
                        elem_step=row,
                        single_packet=False,
                    )
                    nc.gpsimd.dma_gather(
                        hx[:, ctlo:ct, :],
                        tab_full[SPLIT:NPAD],
                        cblob[:, offs["hi"] + hi_coff[ci] : offs["hi"] + hi_coff[ci] + cthi * P // 16],
                        cthi * P,
                        kregs[cthi * P],
                        row,
                        elem_step=row,
                        single_packet=False,
                    )
                    # er for each slot's 128 dsts
                    er_blks = {}
                    for j in ch:
                        erch = sp.tile([P, 128], bf16, tag="erch")
                        nc.sync.dma_start(
                            erch[:], tab_shard[j * P : (j + 1) * P, row - 128 : row]
                        )
                        eroff0 = 128 - (row - hsz)
                        er_blk = sp.tile([P, heads], bf16, tag=f"er_blk{heads}")
                        nc.scalar.activation(
                            er_blk[:],
                            erch[:, eroff0 + 2 * heads : eroff0 + 4 * heads].bitcast(f32),
                            AF.Copy,
                        )
                        er_blks[j] = er_blk
                    # sel one-hot [e, d] for the whole chunk
                    sel = wp.tile([P, CTMAX, P], bf16, tag="sel")
                    do = offs["dstloc"] + dl_coff[ci]
                    nc.vector.tensor_tensor(
                        out=sel[:, 0:ct],
                        in0=cblob[:, do : do + ct]
                        .bitcast(bf16)
                        .unsqueeze(2)
                        .to_broadcast([P, ct, P]),
                        in1=iota.unsqueeze(1).to_broadcast([P, ct, P]),
                        op=ALU.is_equal,
                    )
                    # er per edge via selT matmuls (whole chunk into one PSUM tile)
                    er_ps = pp.tile([P, CTMAX * H], f32, tag="erps", space="PSUM")
                    for j in ch:
                        for t in blk_tiles[j][1]:
                            selT_ps = pp.tile([P, P], bf16, tag="ptr", space="PSUM")
                            nc.tensor.transpose(selT_ps[:], sel[:, t, :], idnb)
                            selT = sp.tile([P, P], bf16, tag="selT_sb")
                            nc.scalar.activation(selT[:], selT_ps[:], AF.Copy)
                            nc.tensor.matmul(
                                er_ps[:, t * heads : (t + 1) * heads],
                                selT[:],
                                er_blks[j][:],
                                start=True,
                                stop=True,
                            )
                    # e = el[src] + er[dst]; w = exp(lrelu(e)); msg = [w | w*h]
                    e_t = sp.tile([P, CTMAX, H], f32, tag="e_t")
                    nc.vector.tensor_tensor(
                        out=e_t[:, 0:ct, 0:heads],
                        in0=hx[:, :, hsz : hsz + 2 * heads].bitcast(f32),
                        in1=er_ps[:, 0 : ct * heads].rearrange("p (t h) -> p t h", h=heads),
                        op=ALU.add,
                    )
                    lr = sp.tile([P, CTMAX, H], f32, tag="lr")
                    nc.vector.tensor_scalar_mul(
                        lr[:, 0:ct, 0:heads], e_t[:, 0:ct, 0:heads], NEG_SLOPE
                    )
                    nc.vector.tensor_tensor(
                        out=lr[:, 0:ct, 0:heads],
                        in0=e_t[:, 0:ct, 0:heads],
                        in1=lr[:, 0:ct, 0:heads],
                        op=ALU.max,
                    )
                    msg = wp.tile([P, CTMAX, nmsg], bf16, tag=f"msg{nmsg}")
                    nc.scalar.activation(msg[:, 0:ct, 0:heads], lr[:, 0:ct, 0:heads], AF.Exp)
                    nc.vector.tensor_tensor(
                        out=msg[:, 0:ct, heads:nmsg],
                        in0=hx[:, :, 0:hsz],
                        in1=msg[:, 0:ct, 0:heads]
                        .unsqueeze(3)
                        .to_broadcast([P, ct, heads, F]),
                        op=ALU.mult,
                    )
                    # per-block segment-reduce + epilogue
                    for j in ch:
                        tls = blk_tiles[j][1]
                        ps_raw = pp.tile([P, H + D], f32, tag="agg", space="PSUM")
                        ps = ps_raw[:, 0:nmsg]
                        for i, t in enumerate(tls):
                            nc.tensor.matmul(
                                ps,
                                sel[:, t, :],
                                msg[:, t, :],
                                start=(i == 0),
                                stop=(i == len(tls) - 1),
                            )
                        rcp = sp.tile([P, 2, heads], f32, tag=f"rcp{heads}")
                        nc.vector.tensor_scalar(
                            out=rcp[:, 0, :], in0=ps[:, 0:heads], scalar1=1e-30,
                            scalar2=None, op0=ALU.max,
                        )
                        nc.vector.reciprocal(rcp[:, 1, :], rcp[:, 0, :])
                        x_sb = sp.tile([P, hsz], bf16, tag=f"x{hsz}")
                        nc.vector.tensor_tensor(
                            out=x_sb[:].rearrange("p (h f) -> p h f", h=heads),
                            in0=ps[:, heads:nmsg].rearrange("p (h f) -> p h f", h=heads),
                            in1=rcp[:, 1, :].unsqueeze(2).to_broadcast([P, heads, F]),
                            op=ALU.mult,
                        )
                        if nxt != "out":
                            nc.vector.tensor_scalar_max(x_sb[:], x_sb[:], 0.0)
                            wall_k, nw, tab_sh_n, row_n, hsz_n = nxt
                            dense_write(x_sb[:], j, wall_k, nw, tab_sh_n, row_n, hsz_n, None)
                        else:
                            xf = sp.tile([P, F], f32, tag="xf")
                            nc.scalar.activation(xf[:], x_sb[:, 0:F], AF.Copy)
                            nc.sync.dma_start(out_ext[j * P : (j + 1) * P, :], xf[:])

            if PHASES >= 3:
                edge_layer(tab1, tab1_sh, ROW, H, D, (wall2_t, NW, tab2_sh, ROW, D))
            if PHASES >= 4:
                nc.gpsimd.collective_compute(
                    "AllGather", ALU.bypass, replica_groups=rg, ins=[tab2_sh[:]], outs=[tab2[:]]
                )
                edge_layer(tab2, tab2_sh, ROW, H, D, (wall3_t, NW3, tab3_sh, ROW3, F))
            if PHASES >= 5:
                nc.gpsimd.collective_compute(
                    "AllGather", ALU.bypass, replica_groups=rg, ins=[tab3_sh[:]], outs=[tab3[:]]
                )
                edge_layer(tab3, tab3_sh, ROW3, 1, F, "out")

    nc.compile()

    in_maps = []
    for c in range(NCORES):
        blob_c = blob0 if c == 0 else blob_for_core(c)[0]
        in_maps.append(
            {
                "cblob": blob_c,
                "featT": np.ascontiguousarray(featT[:, c * SHARD : (c + 1) * SHARD]),
            }
        )

    trace = os.environ.get("GAT_TRACE", "0") == "1"
    if trace and "antenv.axon_hooks" not in sys.modules:
        import types

        _mod = types.ModuleType("antenv.axon_hooks")
        _mod._hook = None
        _mod.set_axon_ntff_profile_hook = lambda h: setattr(_mod, "_hook", h)
        _mod.get_axon_ntff_profile_hook = lambda: _mod._hook
        sys.modules["antenv.axon_hooks"] = _mod
        try:
            from trn_agent_boot.trn_boot import _ntff_profile_via_ctypes

            _mod._hook = _ntff_profile_via_ctypes("/opt/axon/libaxon_pjrt.so")
        except Exception as _e:
            print(f"ntff hook setup failed: {type(_e).__name__}: {_e}")
    res = None
    for attempt in range(4):
        try:
            res = run_bass_kernel_spmd(
                nc, in_maps, list(range(NCORES)), trace=trace and attempt < 2
            )
            break
        except Exception:
            if attempt == 3:
                raise
            import time

            time.sleep(20 * (attempt + 1))
    if trace:
        print(f"HW exec time: {res.exec_time_ns} ns")
        if res.instructions_and_trace is not None:
            print(f"trace path: {res.instructions_and_trace[1]}")
        if res.profile_json is not None:
            print(f"profile json: {res.profile_json}")
        global LAST_RESULTS
        LAST_RESULTS = res
    out_p = np.concatenate([res.results[c]["out"] for c in range(NCORES)], axis=0)
    out = np.ascontiguousarray(out_p[node_perm[np.arange(N)]]).astype(np.float32)
    return out


# revision 11
# speedup vs baseline: 1.5772x; 1.2541x over previous
"""3-layer GAT on 8 trn2 NeuronCores (Bass/Tile).

Strategy (dst-node sharding, v4):
- N padded to 50176 = 392 dst-blocks x 128. Blocks are load-balanced across
  cores (sorted by edge count, rank r -> core r%8, slot r//8); node ids are
  in the permuted core-major shard space.
- Per layer a node table holds per-node rows [h bf16 | el f32 | er f32]
  (768 B for 256-dim layers, 256 B for the 64-dim layer-3 input). The dense
  phase is sharded (bf16 matmuls on PE); one AllGather replicates the table.
- Edge phase per block: two dma_gathers pull src rows (lo/hi split from the
  int16 index limit), per-slot tile counts baked as max over the 8 cores.
  The one-hot Sel/SelT matrices are HOST-BUILT constants streamed by plain
  DMA (no on-device build).  Per edge: e = el_src + er_dst (er expanded per
  edge by a SelT matmul), w = exp(leaky_relu(e)) on the scalar engine, which
  also pre-broadcasts w across the 64 feature cols; one in-place vector mult
  forms [w | w*h] and a Sel matmul segment-reduces it into PSUM (softmax
  max-subtraction cancels in the num/den ratio; logits are O(1)).
- The next layer's dense matmul for each block is interleaved after the
  block epilogue so it hides inside the gather stream.
"""

import os
import sys

sys.path.insert(0, "/opt/trn_rl_repo")

PHASES = int(os.environ.get("GAT_PHASES", "99"))

import numpy as np
import ml_dtypes

import concourse.bass as bass
import concourse.tile as tile
import concourse.mybir as mybir
from concourse import bacc
from concourse.bass_utils import run_bass_kernel_spmd

bf16 = mybir.dt.bfloat16
f32 = mybir.dt.float32
i16 = mybir.dt.int16
AF = mybir.ActivationFunctionType
ALU = mybir.AluOpType

NCORES = 8
P = 128
SPLIT = 32768
NEG_SLOPE = 0.2
H = 4
F = 64
D = H * F  # 256
ROW = 384  # bf16 cols per 256-dim table row (h 0:256 | el f32 | er f32 | pad)
ROW3 = 128  # bf16 cols per 64-dim table row


def _wrap_idx(arr):
    """[K] int16 (K%128==0) -> [128, K//16] dma_gather index layout."""
    k = arr.shape[0]
    a = arr.reshape(k // 16, 16).T
    return np.ascontiguousarray(np.tile(a, (8, 1)))


def _block_diag(a):
    h, f = a.shape
    out = np.zeros((h * f, h), np.float32)
    for i in range(h):
        out[i * f : (i + 1) * f, i] = a[i]
    return out


def kernel(feat, src, dst, W1, al1, ar1, b1, W2, al2, ar2, b2, W3, al3, ar3, b3):
    feat = np.asarray(feat, np.float32)
    src = np.asarray(src).astype(np.int64)
    dst = np.asarray(dst).astype(np.int64)
    params = [np.asarray(p, np.float32) for p in (W1, al1, ar1, b1, W2, al2, ar2, b2, W3, al3, ar3, b3)]
    W1, al1, ar1, b1, W2, al2, ar2, b2, W3, al3, ar3, b3 = params
    assert abs(b1).max() == 0 and abs(b2).max() == 0 and abs(b3).max() == 0

    N, DIN = feat.shape
    assert DIN == P
    NBLK = -(-(-(-N // P)) // NCORES) * NCORES  # 392
    NPAD = NBLK * P
    BPC = NBLK // NCORES  # 49
    SHARD = BPC * P  # 6272

    # ---- host: block -> (core, slot) balanced assignment ----
    oblk = dst // P
    ocounts = np.bincount(oblk, minlength=NBLK)
    rank = np.argsort(-ocounts, kind="stable")
    blk_core = np.empty(NBLK, np.int64)
    blk_slot = np.empty(NBLK, np.int64)
    blk_core[rank] = np.arange(NBLK) % NCORES
    blk_slot[rank] = np.arange(NBLK) // NCORES
    nodes = np.arange(NPAD, dtype=np.int64)
    node_perm = blk_core[nodes // P] * SHARD + blk_slot[nodes // P] * P + nodes % P
    src_p = node_perm[src]
    ecore = blk_core[oblk]
    eslot = blk_slot[oblk]
    dloc = dst % P

    lo_lists = [[None] * BPC for _ in range(NCORES)]
    hi_lists = [[None] * BPC for _ in range(NCORES)]
    order = np.lexsort((src_p, eslot, ecore))
    src_s, dl_s = src_p[order], dloc[order]
    ec_s, es_s = ecore[order], eslot[order]
    bounds = np.searchsorted(ec_s * BPC + es_s, np.arange(NCORES * BPC + 1))
    for c in range(NCORES):
        for j in range(BPC):
            s, e = bounds[c * BPC + j], bounds[c * BPC + j + 1]
            ss, dd = src_s[s:e], dl_s[s:e]
            nl = int(np.searchsorted(ss, SPLIT))
            lo_lists[c][j] = (ss[:nl].astype(np.int16), dd[:nl])
            hi_lists[c][j] = ((ss[nl:] - SPLIT).astype(np.int16), dd[nl:])

    tlo = [max(max(-(-len(lo_lists[c][j][0]) // P), 1) for c in range(NCORES)) for j in range(BPC)]
    thi = [max(max(-(-len(hi_lists[c][j][0]) // P), 1) for c in range(NCORES)) for j in range(BPC)]
    T = [tlo[j] + thi[j] for j in range(BPC)]
    TMAX = max(T)
    TOT = sum(T)
    t_off = np.zeros(BPC + 1, np.int64)
    np.cumsum(T, out=t_off[1:])

    # ---- host: idx constants + sel/selT one-hot constants ----
    def core_consts(c):
        lo_cols, hi_cols = [], []
        sel_np = np.zeros((P, TOT, P), np.int16)
        selT_np = np.zeros((P, TOT, P), np.int16)
        one = np.float32(1.0).astype(ml_dtypes.bfloat16).view(np.int16)
        for j in range(BPC):
            a = np.zeros(tlo[j] * P, np.int16)
            v = lo_lists[c][j][0]
            a[: len(v)] = v
            lo_cols.append(_wrap_idx(a))
            a = np.zeros(thi[j] * P, np.int16)
            v = hi_lists[c][j][0]
            a[: len(v)] = v
            hi_cols.append(_wrap_idx(a))
            dl = np.full(T[j] * P, -1, np.int64)
            dlo, dhi = lo_lists[c][j][1], hi_lists[c][j][1]
            dl[: len(dlo)] = dlo
            dl[tlo[j] * P : tlo[j] * P + len(dhi)] = dhi
            dlr = dl.reshape(T[j], P)  # [t, e]
            oh = (dlr[:, :, None] == np.arange(P)[None, None, :])  # [t, e, d]
            t0 = t_off[j]
            sel_np[:, t0 : t0 + T[j], :] = oh.transpose(1, 0, 2) * one
            selT_np[:, t0 : t0 + T[j], :] = oh.transpose(2, 0, 1) * one
        return (
            np.concatenate(lo_cols, axis=1),
            np.concatenate(hi_cols, axis=1),
            np.ascontiguousarray(sel_np.reshape(P, TOT * P)),
            np.ascontiguousarray(selT_np.reshape(P, TOT * P)),
        )

    # ---- host: weights (bf16) ----
    def wall(W, al, ar):
        wel = W @ _block_diag(al)
        wer = W @ _block_diag(ar)
        return (
            np.concatenate([W, wel, wer], axis=1)
            .astype(ml_dtypes.bfloat16)
            .view(np.int16)
        )

    wall1 = wall(W1, al1, ar1)
    wall2 = wall(W2, al2, ar2)
    wall3 = wall(W3, al3, ar3)
    NW = D + 2 * H  # 264
    NW3 = F + 2  # 66

    idnb_np = np.eye(P, dtype=np.float32).astype(ml_dtypes.bfloat16)

    def blob_for_core(c):
        lo_i, hi_i, sel_i, selT_i = core_consts(c)
        fields = [
            ("idnb", idnb_np.view(np.int16)),
            ("wall1", wall1),
            ("wall2k0", wall2[0:P]),
            ("wall2k1", wall2[P : 2 * P]),
            ("wall3k0", wall3[0:P]),
            ("wall3k1", wall3[P : 2 * P]),
            ("lo", lo_i),
            ("hi", hi_i),
        ]
        blob = np.concatenate([f for _, f in fields], axis=1)
        if blob.shape[1] % 2:
            blob = np.concatenate([blob, np.zeros((P, 1), np.int16)], axis=1)
        offs = {}
        o = 0
        for name, f in fields:
            offs[name] = o
            o += f.shape[1]
        return np.ascontiguousarray(blob), offs, sel_i, selT_i

    blob0, offs, sel0, selT0 = blob_for_core(0)
    CB = blob0.shape[1]

    lo_coff = np.zeros(BPC + 1, np.int64)
    hi_coff = np.zeros(BPC + 1, np.int64)
    np.cumsum([tlo[j] * P // 16 for j in range(BPC)], out=lo_coff[1:])
    np.cumsum([thi[j] * P // 16 for j in range(BPC)], out=hi_coff[1:])

    featT = np.zeros((P, NPAD), np.float32)
    featT[:, node_perm[np.arange(N)]] = feat.T
    featT = featT.astype(ml_dtypes.bfloat16)

    # ---- build program ----
    nc = bacc.Bacc("TRN2", target_bir_lowering=False, debug=False, num_devices=NCORES)

    cblob_in = nc.dram_tensor("cblob", [P, CB], i16, kind="ExternalInput")
    sel_in = nc.dram_tensor("selc", [P, TOT * P], bf16, kind="ExternalInput")
    selT_in = nc.dram_tensor("selTc", [P, TOT * P], bf16, kind="ExternalInput")
    featT_in = nc.dram_tensor("featT", [P, SHARD], bf16, kind="ExternalInput")
    out_ext = nc.dram_tensor("out", [SHARD, F], f32, kind="ExternalOutput")

    tab1_sh = nc.dram_tensor("tab1_sh", [SHARD, ROW], bf16)
    tab2_sh = nc.dram_tensor("tab2_sh", [SHARD, ROW], bf16)
    tab3_sh = nc.dram_tensor("tab3_sh", [SHARD, ROW3], bf16)
    tab1 = nc.dram_tensor("tab1", [NPAD, ROW], bf16, addr_space="Shared")
    tab2 = nc.dram_tensor("tab2", [NPAD, ROW], bf16, addr_space="Shared")
    tab3 = nc.dram_tensor("tab3", [NPAD, ROW3], bf16, addr_space="Shared")

    rg = [list(range(NCORES))]

    with tile.TileContext(nc) as tc:
        with (
            tc.tile_pool(name="const", bufs=1) as cp,
            tc.tile_pool(name="hxp", bufs=3) as hxp,
            tc.tile_pool(name="selp", bufs=2) as selp,
            tc.tile_pool(name="work", bufs=2) as wp,
            tc.tile_pool(name="small", bufs=2) as sp,
            tc.tile_pool(name="psum", bufs=2, space="PSUM") as pp,
        ):
            cblob = cp.tile([P, CB], i16)
            nc.sync.dma_start(cblob[:], cblob_in[:])
            idnb = cblob[:, offs["idnb"] : offs["idnb"] + P].bitcast(bf16)
            wall1_t = cblob[:, offs["wall1"] : offs["wall1"] + NW].bitcast(bf16)
            wall2_t = [
                cblob[:, offs[f"wall2k{k}"] : offs[f"wall2k{k}"] + NW].bitcast(bf16)
                for k in range(2)
            ]
            wall3_t = [
                cblob[:, offs[f"wall3k{k}"] : offs[f"wall3k{k}"] + NW3].bitcast(bf16)
                for k in range(2)
            ]

            kregs = {}
            for j in range(BPC):
                for k in (tlo[j] * P, thi[j] * P):
                    if k not in kregs:
                        kregs[k] = nc.gpsimd.to_reg(k)

            def dense_write(x_ap, j, wall_k, nw, tab_shard, row_cols, hsz, first):
                psd = pp.tile([P, NW], f32, tag="psd", space="PSUM")
                nk = len(wall_k)
                if first is not None:
                    nc.tensor.matmul(psd[:, :nw], first, wall_k[0][:, :nw], start=True, stop=True)
                else:
                    lhsT = sp.tile([P, 2, P], bf16, tag="lhsT")
                    for k in range(nk):
                        ptr = pp.tile([P, P], bf16, tag="ptr", space="PSUM")
                        nc.tensor.transpose(ptr[:], x_ap[:, k * P : (k + 1) * P], idnb)
                        nc.vector.tensor_copy(lhsT[:, k, :], ptr[:])
                    for k in range(nk):
                        nc.tensor.matmul(
                            psd[:, :nw],
                            lhsT[:, k, :],
                            wall_k[k][:, :nw],
                            start=(k == 0),
                            stop=(k == nk - 1),
                        )
                row = sp.tile([P, ROW], bf16, tag="row")
                nc.vector.tensor_copy(row[:, 0:hsz], psd[:, 0:hsz])
                nc.vector.tensor_copy(
                    row[:, hsz : hsz + 2 * (nw - hsz)].bitcast(f32),
                    psd[:, hsz:nw],
                )
                nc.sync.dma_start(tab_shard[j * P : (j + 1) * P, :], row[:, 0:row_cols])

            # ---- dense layer 1 ----
            for j in range(BPC):
                ft = sp.tile([P, P], bf16, tag="ft")
                nc.sync.dma_start(ft[:], featT_in[:, j * P : (j + 1) * P])
                dense_write(None, j, [wall1_t], NW, tab1_sh, ROW, D, first=ft[:])

            if PHASES >= 2:
                nc.gpsimd.collective_compute(
                    "AllGather", ALU.bypass, replica_groups=rg, ins=[tab1_sh[:]], outs=[tab1[:]]
                )

            # ---- edge phase for one layer ----
            def edge_layer(tab_full, tab_shard, row, heads, hsz, nxt):
                nmsg = heads + hsz
                for j in range(BPC):
                    tl, th, tj = tlo[j], thi[j], T[j]
                    hx_raw = hxp.tile([P, TMAX * ROW], bf16, tag="hx")
                    hx = hx_raw[:, 0 : tj * row].rearrange("p (t r) -> p t r", r=row)
                    nc.gpsimd.dma_gather(
                        hx[:, 0:tl, :],
                        tab_full[0:SPLIT],
                        cblob[:, offs["lo"] + int(lo_coff[j]) : offs["lo"] + int(lo_coff[j + 1])],
                        tl * P,
                        kregs[tl * P],
                        row,
                        elem_step=row,
                        single_packet=False,
                    )
                    nc.gpsimd.dma_gather(
                        hx[:, tl:tj, :],
                        tab_full[SPLIT:NPAD],
                        cblob[:, offs["hi"] + int(hi_coff[j]) : offs["hi"] + int(hi_coff[j + 1])],
                        th * P,
                        kregs[th * P],
                        row,
                        elem_step=row,
                        single_packet=False,
                    )
                    # sel / selT constants for this block
                    sel = selp.tile([P, TMAX * P], bf16, tag="sel")
                    nc.sync.dma_start(
                        sel[:, 0 : tj * P], sel_in[:, int(t_off[j]) * P : int(t_off[j] + tj) * P]
                    )
                    selT = selp.tile([P, TMAX * P], bf16, tag="selT")
                    nc.sync.dma_start(
                        selT[:, 0 : tj * P], selT_in[:, int(t_off[j]) * P : int(t_off[j] + tj) * P]
                    )
                    # er for the block's 128 dsts
                    erch = sp.tile([P, 128], bf16, tag="erch")
                    nc.sync.dma_start(
                        erch[:], tab_shard[j * P : (j + 1) * P, row - 128 : row]
                    )
                    eroff0 = 128 - (row - hsz)
                    er_blk = sp.tile([P, heads], bf16, tag=f"er_blk{heads}")
                    nc.scalar.activation(
                        er_blk[:],
                        erch[:, eroff0 + 2 * heads : eroff0 + 4 * heads].bitcast(f32),
                        AF.Copy,
                    )
                    # er per edge via selT matmuls
                    er_ps = pp.tile([P, TMAX * H], f32, tag="erps", space="PSUM")
                    for t in range(tj):
                        nc.tensor.matmul(
                            er_ps[:, t * heads : (t + 1) * heads],
                            selT[:, t * P : (t + 1) * P],
                            er_blk[:],
                            start=True,
                            stop=True,
                        )
                    # e = el + er; w = exp(lrelu(e)); msg = [w | w*h]
                    e_t = sp.tile([P, TMAX * H], f32, tag="e_t")
                    nc.vector.tensor_tensor(
                        out=e_t[:, 0 : tj * heads].rearrange("p (t h) -> p t h", h=heads),
                        in0=hx[:, :, hsz : hsz + 2 * heads].bitcast(f32),
                        in1=er_ps[:, 0 : tj * heads].rearrange("p (t h) -> p t h", h=heads),
                        op=ALU.add,
                    )
                    lr = sp.tile([P, TMAX * H], f32, tag="lr")
                    nc.vector.tensor_scalar_mul(
                        lr[:, 0 : tj * heads], e_t[:, 0 : tj * heads], NEG_SLOPE
                    )
                    nc.vector.tensor_tensor(
                        out=lr[:, 0 : tj * heads],
                        in0=e_t[:, 0 : tj * heads],
                        in1=lr[:, 0 : tj * heads],
                        op=ALU.max,
                    )
                    msg = wp.tile([P, TMAX, nmsg], bf16, tag=f"msg{nmsg}")
                    nc.scalar.activation(
                        msg[:, 0:tj, 0:heads],
                        lr[:, 0 : tj * heads].rearrange("p (t h) -> p t h", h=heads),
                        AF.Exp,
                    )
                    # broadcast w across the 64 feature cols (scalar engine)
                    nc.scalar.activation(
                        msg[:, 0:tj, heads:nmsg].rearrange("p t (h f) -> p t h f", f=F),
                        msg[:, 0:tj, 0:heads].unsqueeze(3).to_broadcast([P, tj, heads, F]),
                        AF.Copy,
                    )
                    # msg *= h  (in place)
                    nc.vector.tensor_tensor(
                        out=msg[:, 0:tj, heads:nmsg],
                        in0=msg[:, 0:tj, heads:nmsg],
                        in1=hx[:, :, 0:hsz],
                        op=ALU.mult,
                    )
                    # segment-reduce into PSUM
                    ps_raw = pp.tile([P, H + D], f32, tag="agg", space="PSUM")
                    ps = ps_raw[:, 0:nmsg]
                    for t in range(tj):
                        nc.tensor.matmul(
                            ps,
                            sel[:, t * P : (t + 1) * P],
                            msg[:, t, :],
                            start=(t == 0),
                            stop=(t == tj - 1),
                        )
                    # epilogue: out = act(wh_sum / w_sum)
                    rcp = sp.tile([P, 2, heads], f32, tag=f"rcp{heads}")
                    nc.vector.tensor_scalar(
                        out=rcp[:, 0, :], in0=ps[:, 0:heads], scalar1=1e-30,
                        scalar2=None, op0=ALU.max,
                    )
                    nc.vector.reciprocal(rcp[:, 1, :], rcp[:, 0, :])
                    x_sb = sp.tile([P, hsz], bf16, tag=f"x{hsz}")
                    nc.vector.tensor_tensor(
                        out=x_sb[:].rearrange("p (h f) -> p h f", h=heads),
                        in0=ps[:, heads:nmsg].rearrange("p (h f) -> p h f", h=heads),
                        in1=rcp[:, 1, :].unsqueeze(2).to_broadcast([P, heads, F]),
                        op=ALU.mult,
                    )
                    if nxt != "out":
                        nc.vector.tensor_scalar_max(x_sb[:], x_sb[:], 0.0)
                        wall_k, nw, tab_sh_n, row_n, hsz_n = nxt
                        dense_write(x_sb[:], j, wall_k, nw, tab_sh_n, row_n, hsz_n, None)
                    else:
                        xf = sp.tile([P, F], f32, tag="xf")
                        nc.scalar.activation(xf[:], x_sb[:, 0:F], AF.Copy)
                        nc.sync.dma_start(out_ext[j * P : (j + 1) * P, :], xf[:])

            if PHASES >= 3:
                edge_layer(tab1, tab1_sh, ROW, H, D, (wall2_t, NW, tab2_sh, ROW, D))
            if PHASES >= 4:
                nc.gpsimd.collective_compute(
                    "AllGather", ALU.bypass, replica_groups=rg, ins=[tab2_sh[:]], outs=[tab2[:]]
                )
                edge_layer(tab2, tab2_sh, ROW, H, D, (wall3_t, NW3, tab3_sh, ROW3, F))
            if PHASES >= 5:
                nc.gpsimd.collective_compute(
                    "AllGather", ALU.bypass, replica_groups=rg, ins=[tab3_sh[:]], outs=[tab3[:]]
                )
                edge_layer(tab3, tab3_sh, ROW3, 1, F, "out")

    nc.compile()

    in_maps = []
    for c in range(NCORES):
        if c == 0:
            blob_c, sel_c, selT_c = blob0, sel0, selT0
        else:
            blob_c, _, sel_c, selT_c = blob_for_core(c)
        in_maps.append(
            {
                "cblob": blob_c,
                "selc": sel_c.view(ml_dtypes.bfloat16),
                "selTc": selT_c.view(ml_dtypes.bfloat16),
                "featT": np.ascontiguousarray(featT[:, c * SHARD : (c + 1) * SHARD]),
            }
        )

    trace = os.environ.get("GAT_TRACE", "0") == "1"
    if trace and "antenv.axon_hooks" not in sys.modules:
        import types

        _mod = types.ModuleType("antenv.axon_hooks")
        _mod._hook = None
        _mod.set_axon_ntff_profile_hook = lambda h: setattr(_mod, "_hook", h)
        _mod.get_axon_ntff_profile_hook = lambda: _mod._hook
        sys.modules["antenv.axon_hooks"] = _mod
        try:
            from trn_agent_boot.trn_boot import _ntff_profile_via_ctypes

            _mod._hook = _ntff_profile_via_ctypes("/opt/axon/libaxon_pjrt.so")
        except Exception as _e:
            print(f"ntff hook setup failed: {type(_e).__name__}: {_e}")
    res = None
    for attempt in range(4):
        try:
            res = run_bass_kernel_spmd(
                nc, in_maps, list(range(NCORES)), trace=trace and attempt < 2
            )
            break
        except Exception:
            if attempt == 3:
                raise
            import time

            time.sleep(20 * (attempt + 1))
    if trace:
        print(f"HW exec time: {res.exec_time_ns} ns")
        if res.instructions_and_trace is not None:
            print(f"trace path: {res.instructions_and_trace[1]}")
        if res.profile_json is not None:
            print(f"profile json: {res.profile_json}")
        global LAST_RESULTS
        LAST_RESULTS = res
    out_p = np.concatenate([res.results[c]["out"] for c in range(NCORES)], axis=0)
    out = np.ascontiguousarray(out_p[node_perm[np.arange(N)]]).astype(np.float32)
    return out
